# revision 4
# baseline (speedup 1.0000x reference)
"""Trainium2 Bass kernel for nn_BidirectionalRNN (3-layer LN-tanh RNN, bidir).

Sharding: 8 cores = 4 batch-shards x 2 directions (l2r on cores 0-3, r2l on
cores 4-7), B_loc=32 batches per core. All cores run the same SPMD program;
per-core inputs differ (direction weights + batch slice).

On-core layout: everything "transposed" — H on partitions as 4 chunks of 128,
batch along free dim. State h_l is one SBUF tile [128, 4, 32] (chunk-major).
Per step/layer:
  psum_pre[128,4,32] = Wh_l.T @ h_l(t-1) + Wx_l.T @ h_{l-1}(t)   (bf16 matmuls)
  s  = psum_pre + bias  (layer0: + xproj[t], bias prefolded)  -> bf16 st tile
  s2 = s*s                                                     -> st cols B:2B
  stats[1,2B] = (ones/512).T @ st  (PE, per k-chunk accumulate)  = [mean|meansq]
  m2 = Square(mean) (ACT); var = meansq - m2 (DVE)
  rstd = pow(var + eps, -0.5) (DVE tensor_scalar dual-op); c = mean*rstd
  [A|C][128,2B] = ones16.T @ [rstd|c]  (PE broadcast, fp16)
  y = s*A - C ; h_l = Tanh(y) (ACT, out bf16)
Embedding + xproj0 for layer0 are precomputed into SBUF (xp tile, bf16) by a
slab-wise pre-phase: xemb.T = Wemb_aug.T @ xT_aug (bias via appended ones row),
xproj = Wx0.T @ xemb.T + (bx0+bh0).
Final FC is accumulated per step on the PE: L[45, t*B:(t+1)*B] = Wfc_half.T @ h2
(+ b_fc on the l2r cores only, via per-core input).
Host combines: logits[b,t] = L_l2r[b,t] + L_r2l[b, idx[b,t]] (time gather
commutes with the channel-wise FC).
"""

import numpy as np
import ml_dtypes

import concourse.bass as bass
import concourse.bacc as bacc
import concourse.tile as tile
from concourse import mybir
from concourse.bass_utils import run_bass_kernel_spmd

BF16 = ml_dtypes.bfloat16
import os
USE_FP16 = os.environ.get("KERNEL_DT16", "f16") == "f16"
NP16 = np.float16 if USE_FP16 else BF16

H = 512
IN_DIM = 300
NCLS = 45
LN_EPS = 1e-5
P = 128
KC = H // P  # 4 chunks
N_CORES = 8

f32 = mybir.dt.float32
f16 = mybir.dt.float16
bf16 = mybir.dt.float16 if USE_FP16 else mybir.dt.bfloat16


def _stride0_view(ap, reps, width):
    """[P, width] AP -> [P, reps, width] AP re-reading the same cols."""
    return bass.AP(tensor=ap.tensor, offset=ap.offset,
                   ap=[ap.ap[0], [0, reps], [1, width]])


def build_nc(T=256, B=32, apply_gb=False):
    """Build the SPMD program. B = batches per core."""
    COLS = T * B
    S = min(1024, COLS)          # pre-phase slab width (cols)
    MMN = min(512, S)            # matmul moving width
    n_slabs = COLS // S

    nc = bacc.Bacc(None, target_bir_lowering=False)

    # ---- DRAM parameters (per-core values supplied via in_maps) ----
    xt_d = nc.dram_tensor("xt", [3, P, COLS], bf16, kind="ExternalInput")
    wemb_d = nc.dram_tensor("wemb", [P, 3, H], bf16, kind="ExternalInput")
    wx0_d = nc.dram_tensor("wx0", [P, KC, H], bf16, kind="ExternalInput")
    # recurrence weights: Wh0, Wx1, Wh1, Wx2, Wh2
    wrec_d = nc.dram_tensor("wrec", [5, P, KC, H], bf16, kind="ExternalInput")
    bias0_d = nc.dram_tensor("bias0", [P, KC], f32, kind="ExternalInput")
    bias12_d = nc.dram_tensor("bias12", [P, 2, KC], f32, kind="ExternalInput")
    wfc_d = nc.dram_tensor("wfc", [P, KC, NCLS], bf16, kind="ExternalInput")
    fcb_d = nc.dram_tensor("fcb", [NCLS, 1], f32, kind="ExternalInput")
    if apply_gb:
        gb_d = nc.dram_tensor("gb", [P, 3, 2, KC], f32, kind="ExternalInput")
    out_d = nc.dram_tensor("out", [NCLS, COLS], f32, kind="ExternalOutput")

    with tile.TileContext(nc) as tc:
        import contextlib
        with contextlib.ExitStack() as ctx:
            const = ctx.enter_context(tc.tile_pool(name="const", bufs=1))
            big = ctx.enter_context(tc.tile_pool(name="big", bufs=1))
            xtp = ctx.enter_context(tc.tile_pool(name="xtp", bufs=2))
            xep = ctx.enter_context(tc.tile_pool(name="xep", bufs=2))
            stp = ctx.enter_context(tc.tile_pool(name="stp", bufs=3))
            hp = ctx.enter_context(tc.tile_pool(name="hp", bufs=3))
            yp = ctx.enter_context(tc.tile_pool(name="yp", bufs=3))
            tiny = ctx.enter_context(tc.tile_pool(name="tiny", bufs=4))
            ps_pre = ctx.enter_context(tc.tile_pool(name="ps_pre", bufs=2, space="PSUM"))
            ps_st = ctx.enter_context(tc.tile_pool(name="ps_st", bufs=2, space="PSUM"))
            ps_bc = ctx.enter_context(tc.tile_pool(name="ps_bc", bufs=2, space="PSUM"))
            ps_l = ctx.enter_context(tc.tile_pool(name="ps_l", bufs=1, space="PSUM"))
            ps_bp = ctx.enter_context(tc.tile_pool(name="ps_bp", bufs=1, space="PSUM"))

            # ---- constants / weights into SBUF ----
            wemb_sb = const.tile([P, 3, H], bf16)
            nc.sync.dma_start(out=wemb_sb, in_=wemb_d.ap())
            wx0_sb = const.tile([P, KC, H], bf16)
            nc.sync.dma_start(out=wx0_sb, in_=wx0_d.ap())
            wrec_sb = const.tile([P, 5, KC, H], bf16)
            nc.sync.dma_start(out=wrec_sb, in_=wrec_d.ap().rearrange("n p k m -> p n k m"))
            bias0_sb = const.tile([P, KC], f32)
            nc.sync.dma_start(out=bias0_sb, in_=bias0_d.ap())
            bias12_sb = const.tile([P, 2, KC], f32)
            nc.sync.dma_start(out=bias12_sb, in_=bias12_d.ap())
            wfc_sb = const.tile([P, KC, NCLS], bf16)
            nc.sync.dma_start(out=wfc_sb, in_=wfc_d.ap())
            fcb_sb = const.tile([NCLS, 1], f32)
            nc.sync.dma_start(out=fcb_sb, in_=fcb_d.ap())
            if apply_gb:
                gb_sb = const.tile([P, 3, 2, KC], f32)
                nc.sync.dma_start(out=gb_sb, in_=gb_d.ap())

            ones16 = const.tile([1, P], f16)
            nc.vector.memset(ones16, 1.0)
            sc_ones = const.tile([P, 1], bf16)
            nc.vector.memset(sc_ones, 1.0 / H)
            qk_sb = const.tile([1, 512], mybir.dt.int32)
            nc.vector.memset(qk_sb, 0x5F3759DF)

            xp_sb = big.tile([P, T, KC, B], bf16)     # xproj0 (+bias0), all steps
            L_sb = big.tile([NCLS, COLS], f32)        # FC accumulator

            # ---- pre-phase: embedding + xproj0, slab by slab ----
            for sl in range(n_slabs):
                c0 = sl * S
                xt_tiles = []
                for k in range(3):
                    xt_t = xtp.tile([P, S], bf16, tag=f"xt{k}")
                    nc.sync.dma_start(out=xt_t, in_=xt_d.ap()[k, :, c0:c0 + S])
                    xt_tiles.append(xt_t)
                xe_tiles = []
                for m in range(KC):
                    xe_t = xep.tile([P, S], bf16, tag=f"xe{m}")
                    xe_tiles.append(xe_t)
                for m in range(KC):
                    for ns in range(S // MMN):
                        pse = ps_bp.tile([P, MMN], f32, tag="bp")
                        for k in range(3):
                            nc.tensor.matmul(pse, wemb_sb[:, k, bass.ts(m, P)],
                                             xt_tiles[k][:, bass.ts(ns, MMN)],
                                             start=(k == 0), stop=(k == 2))
                        nc.scalar.copy(xe_tiles[m][:, bass.ts(ns, MMN)], pse)
                for m in range(KC):
                    for ns in range(S // MMN):
                        psx = ps_bp.tile([P, MMN], f32, tag="bp")
                        for k in range(KC):
                            nc.tensor.matmul(psx, wx0_sb[:, k, bass.ts(m, P)],
                                             xe_tiles[k][:, bass.ts(ns, MMN)],
                                             start=(k == 0), stop=(k == KC - 1))
                        n0 = c0 + ns * MMN
                        t0 = n0 // B
                        nt = MMN // B
                        nc.vector.tensor_scalar(
                            xp_sb[:, t0:t0 + nt, m, :], psx,
                            bias0_sb[:, m:m + 1], None, mybir.AluOpType.add)

            # ---- recurrence ----
            h = []
            for l in range(3):
                h0 = hp.tile([P, KC, B], bf16, tag=f"h{l}")
                nc.vector.memset(h0, 0.0)
                h.append(h0)

            wh_idx = [0, 2, 4]   # Wh0, Wh1, Wh2 in wrec
            wx_idx = [None, 1, 3]

            for t in range(T):
                for l in range(3):
                    ps = ps_pre.tile([P, KC, B], f32, tag="pre")
                    for m in range(KC):
                        ops = [(wh_idx[l], h[l])]
                        if l > 0:
                            ops.append((wx_idx[l], h[l - 1]))
                        n_mm = len(ops) * KC
                        i = 0
                        for wsel, hsrc in ops:
                            for k in range(KC):
                                nc.tensor.matmul(
                                    ps[:, m, :],
                                    wrec_sb[:, wsel, k, bass.ts(m, P)],
                                    hsrc[:, k, :],
                                    start=(i == 0), stop=(i == n_mm - 1))
                                i += 1
                    st = stp.tile([P, KC, 2 * B], bf16, tag="st")
                    if l == 0:
                        src1 = xp_sb[:, t, :, :]
                    else:
                        src1 = _stride0_view_mid(bias12_sb[:, l - 1, :], B)
                    nc.vector.tensor_tensor(st[:, :, :B], ps, src1,
                                            mybir.AluOpType.add)
                    nc.vector.tensor_tensor(st[:, :, B:], st[:, :, :B],
                                            st[:, :, :B], mybir.AluOpType.mult)
                    pst = ps_st.tile([1, 2 * B], f32, tag="pst")
                    for k in range(KC):
                        nc.tensor.matmul(pst, sc_ones, st[:, k, :],
                                         start=(k == 0), stop=(k == KC - 1))
                    m2 = tiny.tile([1, B], f32, tag="m2")
                    nc.scalar.activation(m2, pst[:, :B],
                                         mybir.ActivationFunctionType.Square)
                    ve = tiny.tile([1, B], f32, tag="ve")
                    nc.vector.tensor_tensor(ve, pst[:, B:], m2,
                                            mybir.AluOpType.subtract)
                    nc.vector.tensor_scalar(ve, ve, LN_EPS, None,
                                            mybir.AluOpType.add)
                    # rsqrt via quake bit-trick + 2 Newton iterations (DVE)
                    ui = tiny.tile([1, B], mybir.dt.int32, tag="ui")
                    nc.vector.tensor_scalar(ui, ve.bitcast(mybir.dt.int32), 1,
                                            None, mybir.AluOpType.arith_shift_right)
                    y0i = tiny.tile([1, B], mybir.dt.int32, tag="y0i")
                    nc.vector.tensor_tensor(y0i, qk_sb[:, :B], ui,
                                            mybir.AluOpType.subtract)
                    cur = y0i.bitcast(f32)
                    ac = tiny.tile([1, 2 * B], f16, tag="ac")
                    for it in range(2):
                        y2 = tiny.tile([1, B], f32, tag=f"nw_y2_{it}")
                        nc.vector.tensor_tensor(y2, cur, cur, mybir.AluOpType.mult)
                        xy2 = tiny.tile([1, B], f32, tag=f"nw_xy2_{it}")
                        nc.vector.tensor_tensor(xy2, ve, y2, mybir.AluOpType.mult)
                        e = tiny.tile([1, B], f32, tag=f"nw_e_{it}")
                        nc.vector.tensor_scalar(e, xy2, -0.5, 1.5,
                                                mybir.AluOpType.mult,
                                                mybir.AluOpType.add)
                        if it == 0:
                            yn = tiny.tile([1, B], f32, tag="nw_yn")
                            nc.vector.tensor_tensor(yn, cur, e, mybir.AluOpType.mult)
                            cur = yn
                        else:
                            nc.vector.tensor_tensor(ac[:, :B], cur, e,
                                                    mybir.AluOpType.mult)
                    nc.vector.tensor_tensor(ac[:, B:], pst[:, :B], ac[:, :B],
                                            mybir.AluOpType.mult)
                    pbc = ps_bc.tile([P, 2 * B], f32, tag="bc")
                    nc.tensor.matmul(pbc, ones16, ac, start=True, stop=True)
                    y = yp.tile([P, KC, B], f32, tag="y")
                    nc.vector.tensor_tensor(y, st[:, :, :B],
                                            _stride0_view(pbc[:, :B], KC, B),
                                            mybir.AluOpType.mult)
                    nc.vector.tensor_tensor(y, y,
                                            _stride0_view(pbc[:, B:], KC, B),
                                            mybir.AluOpType.subtract)
                    if apply_gb:
                        nc.vector.tensor_tensor(
                            y, y, _stride0_view_mid(gb_sb[:, l, 0, :], B),
                            mybir.AluOpType.mult)
                        nc.vector.tensor_tensor(
                            y, y, _stride0_view_mid(gb_sb[:, l, 1, :], B),
                            mybir.AluOpType.add)
                    hn = hp.tile([P, KC, B], bf16, tag=f"h{l}")
                    nc.scalar.activation(hn, y, mybir.ActivationFunctionType.Tanh)
                    h[l] = hn
                psl = ps_l.tile([NCLS, B], f32, tag="L")
                for k in range(KC):
                    nc.tensor.matmul(psl, wfc_sb[:, k, :], h[2][:, k, :],
                                     start=(k == 0), stop=(k == KC - 1))
                nc.vector.tensor_scalar(L_sb[:, t * B:(t + 1) * B], psl,
                                        fcb_sb, None, mybir.AluOpType.add)

            nc.sync.dma_start(out=out_d.ap(), in_=L_sb)

    nc.compile()
    return nc


def _stride0_view_mid(ap, width):
    """[P, KC] AP -> [P, KC, width] AP, broadcasting each col along width."""
    return bass.AP(tensor=ap.tensor, offset=ap.offset,
                   ap=[ap.ap[0], ap.ap[1], [0, width]])


# ---------------- host-side prep ----------------

def _lay_w(w):
    """[H, M] fp32 -> [P, KC, M] bf16 chunk layout."""
    Hh, M = w.shape
    kc = Hh // P
    return np.ascontiguousarray(
        w.reshape(kc, P, M).transpose(1, 0, 2)).astype(NP16)


def make_in_maps(inputs, T=256, B=32):
    """Build the 8 per-core input dicts from the full problem inputs."""
    x = np.asarray(inputs["x"], np.float32)[:, :T]
    rx = np.asarray(inputs["reverse_x"], np.float32)[:, :T]
    W_emb = np.asarray(inputs["W_emb"], np.float32)
    b_emb = np.asarray(inputs["b_emb"], np.float32)
    W_fc = np.asarray(inputs["W_fc"], np.float32)
    b_fc = np.asarray(inputs["b_fc"], np.float32)

    wemb_aug = np.zeros((3 * P, H), np.float32)
    wemb_aug[:IN_DIM] = W_emb
    wemb_aug[IN_DIM] = b_emb
    wemb_lay = _lay_w(wemb_aug)  # [P, 3, H]

    dirs = {}
    for d, (xx, sfx, wfc_half, fcb) in enumerate([
            (x, "l2r", W_fc[:H], b_fc),
            (rx, "r2l", W_fc[H:], np.zeros_like(b_fc))]):
        Wx = np.asarray(inputs[f"Wx_{sfx}"], np.float32)
        bx = np.asarray(inputs[f"bx_{sfx}"], np.float32)
        Wh = np.asarray(inputs[f"Wh_{sfx}"], np.float32)
        bh = np.asarray(inputs[f"bh_{sfx}"], np.float32)
        wrec = np.stack([_lay_w(Wh[0]), _lay_w(Wx[1]), _lay_w(Wh[1]),
                         _lay_w(Wx[2]), _lay_w(Wh[2])])  # [5, P, KC, H]
        bias0 = (bx[0] + bh[0]).reshape(KC, P).T.astype(np.float32)  # [P, KC]
        bias12 = np.stack([(bx[1] + bh[1]).reshape(KC, P).T,
                           (bx[2] + bh[2]).reshape(KC, P).T], 1).astype(np.float32)
        dirs[d] = dict(
            x=xx,
            wx0=_lay_w(Wx[0]),
            wrec=np.ascontiguousarray(wrec),
            bias0=np.ascontiguousarray(bias0),
            bias12=np.ascontiguousarray(bias12),
            wfc=_lay_w(wfc_half),
            fcb=fcb.reshape(NCLS, 1).astype(np.float32),
        )

    n_shard = N_CORES // 2
    in_maps = []
    for core in range(N_CORES):
        d = 0 if core < n_shard else 1
        s = core % n_shard
        dd = dirs[d]
        xc = dd["x"][s * B:(s + 1) * B]  # [B, T, IN]
        xa = np.zeros((3 * P, T * B), np.float32)
        xa[:IN_DIM] = xc.transpose(2, 1, 0).reshape(IN_DIM, T * B)
        xa[IN_DIM] = 1.0
        in_maps.append({
            "xt": np.ascontiguousarray(xa.reshape(3, P, T * B)).astype(NP16),
            "wemb": wemb_lay,
            "wx0": dd["wx0"],
            "wrec": dd["wrec"],
            "bias0": dd["bias0"],
            "bias12": dd["bias12"],
            "wfc": dd["wfc"],
            "fcb": dd["fcb"],
        })
    return in_maps


def combine_outputs(results, pad_start_index, T=256, B=32):
    """results: list of 8 dicts with 'out' [NCLS, T*B]. Returns [128*T, NCLS]."""
    n_shard = N_CORES // 2
    Bfull = n_shard * B
    L = np.zeros((2, Bfull, T, NCLS), np.float32)
    for core in range(N_CORES):
        d = 0 if core < n_shard else 1
        s = core % n_shard
        o = results[core]["out"].reshape(NCLS, T, B)  # col = t*B + b
        L[d, s * B:(s + 1) * B] = o.transpose(2, 1, 0)
    p = np.asarray(pad_start_index).astype(np.int64)[:, None]
    j = np.arange(T)[None, :]
    idx = np.where(j < p, p - j - 1, j)  # [Bfull, T]
    L2g = np.take_along_axis(L[1], idx[:, :, None], axis=1)
    logits = L[0] + L2g
    return logits.reshape(Bfull * T, NCLS)


_NC_CACHE = {}


def kernel(**inputs) -> np.ndarray:
    T = int(inputs["max_length"])
    assert T == 256, f"kernel compiled for T=256, got {T}"
    B = 32
    ln_g = np.asarray(inputs["ln_g"], np.float32)
    ln_b = np.asarray(inputs["ln_b"], np.float32)
    apply_gb = not (np.all(ln_g == 1.0) and np.all(ln_b == 0.0))
    assert not apply_gb, "general ln_g/ln_b path not wired yet"

    key = (T, B, apply_gb)
    if key not in _NC_CACHE:
        _NC_CACHE[key] = build_nc(T=T, B=B, apply_gb=apply_gb)
    nc = _NC_CACHE[key]

    in_maps = make_in_maps(inputs, T=T, B=B)
    res = run_bass_kernel_spmd(nc, in_maps, list(range(N_CORES)))
    return combine_outputs(res.results, inputs["pad_start_index"], T=T, B=B)


if __name__ == "__main__":
    import reference
    inp = reference.setup_inputs()
    out = kernel(**{k: np.asarray(v) for k, v in inp.items()})
    ref = np.asarray(reference.reference(**inp))
    err = np.abs(out - ref).max() / np.abs(ref).max()
    print(f"Relative error: {err:.3e}")


# revision 6
# speedup vs baseline: 1.2015x; 1.2015x over previous
"""Trainium2 Bass kernel for nn_BidirectionalRNN (3-layer LN-tanh RNN, bidir).

Sharding: 8 cores = 4 batch-shards x 2 directions (l2r on cores 0-3, r2l on
cores 4-7), B_loc=32 batches per core. All cores run the same SPMD program;
per-core inputs differ (direction weights + batch slice).

On-core layout: everything "transposed" — H on partitions as 4 chunks of 128,
batch along free dim. State h_l is one SBUF tile [128, 4, 32] (chunk-major).
Per step/layer:
  psum_pre[128,4,32] = Wh_l.T @ h_l(t-1) + Wx_l.T @ h_{l-1}(t)   (bf16 matmuls)
  s  = psum_pre + bias  (layer0: + xproj[t], bias prefolded)  -> bf16 st tile
  s2 = s*s                                                     -> st cols B:2B
  stats[1,2B] = (ones/512).T @ st  (PE, per k-chunk accumulate)  = [mean|meansq]
  m2 = Square(mean) (ACT); var = meansq - m2 (DVE)
  rstd = pow(var + eps, -0.5) (DVE tensor_scalar dual-op); c = mean*rstd
  [A|C][128,2B] = ones16.T @ [rstd|c]  (PE broadcast, fp16)
  y = s*A - C ; h_l = Tanh(y) (ACT, out bf16)
Embedding + xproj0 for layer0 are precomputed into SBUF (xp tile, bf16) by a
slab-wise pre-phase: xemb.T = Wemb_aug.T @ xT_aug (bias via appended ones row),
xproj = Wx0.T @ xemb.T + (bx0+bh0).
Final FC is accumulated per step on the PE: L[45, t*B:(t+1)*B] = Wfc_half.T @ h2
(+ b_fc on the l2r cores only, via per-core input).
Host combines: logits[b,t] = L_l2r[b,t] + L_r2l[b, idx[b,t]] (time gather
commutes with the channel-wise FC).
"""

import numpy as np
import ml_dtypes

import concourse.bass as bass
import concourse.bacc as bacc
import concourse.tile as tile
from concourse import mybir
from concourse.bass_utils import run_bass_kernel_spmd

BF16 = ml_dtypes.bfloat16
import os
USE_FP16 = os.environ.get("KERNEL_DT16", "f16") == "f16"
NP16 = np.float16 if USE_FP16 else BF16

H = 512
IN_DIM = 300
NCLS = 45
LN_EPS = 1e-5
P = 128
KC = H // P  # 4 chunks
N_CORES = 8

f32 = mybir.dt.float32
f16 = mybir.dt.float16
bf16 = mybir.dt.float16 if USE_FP16 else mybir.dt.bfloat16


def _stride0_view(ap, reps, width):
    """[P, width] AP -> [P, reps, width] AP re-reading the same cols."""
    return bass.AP(tensor=ap.tensor, offset=ap.offset,
                   ap=[ap.ap[0], [0, reps], [1, width]])


def build_nc(T=256, B=32, apply_gb=False):
    """Build the SPMD program. B = batches per core."""
    COLS = T * B
    S = min(1024, COLS)          # pre-phase slab width (cols)
    MMN = min(512, S)            # matmul moving width
    n_slabs = COLS // S

    nc = bacc.Bacc(None, target_bir_lowering=False)

    # ---- DRAM parameters (per-core values supplied via in_maps) ----
    xt_d = nc.dram_tensor("xt", [3, P, COLS], bf16, kind="ExternalInput")
    wemb_d = nc.dram_tensor("wemb", [P, 3, H], bf16, kind="ExternalInput")
    wx0_d = nc.dram_tensor("wx0", [P, KC, H], bf16, kind="ExternalInput")
    # recurrence weights: Wh0, Wx1, Wh1, Wx2, Wh2
    wrec_d = nc.dram_tensor("wrec", [5, P, KC, H], bf16, kind="ExternalInput")
    bias0_d = nc.dram_tensor("bias0", [P, KC], f32, kind="ExternalInput")
    brow_d = nc.dram_tensor("brow", [1, 2, KC, P], f16, kind="ExternalInput")
    eye_d = nc.dram_tensor("eye32", [B, B], f32, kind="ExternalInput")
    wfc_d = nc.dram_tensor("wfc", [P, KC, NCLS], bf16, kind="ExternalInput")
    fcb_d = nc.dram_tensor("fcb", [NCLS, 1], f32, kind="ExternalInput")
    if apply_gb:
        gb_d = nc.dram_tensor("gb", [P, 3, 2, KC], f32, kind="ExternalInput")
    out_d = nc.dram_tensor("out", [NCLS, COLS], f32, kind="ExternalOutput")

    with tile.TileContext(nc) as tc:
        import contextlib
        with contextlib.ExitStack() as ctx:
            const = ctx.enter_context(tc.tile_pool(name="const", bufs=1))
            big = ctx.enter_context(tc.tile_pool(name="big", bufs=1))
            xtp = ctx.enter_context(tc.tile_pool(name="xtp", bufs=2))
            xep = ctx.enter_context(tc.tile_pool(name="xep", bufs=2))
            stp = ctx.enter_context(tc.tile_pool(name="stp", bufs=3))
            hp = ctx.enter_context(tc.tile_pool(name="hp", bufs=3))
            yp = ctx.enter_context(tc.tile_pool(name="yp", bufs=3))
            tiny = ctx.enter_context(tc.tile_pool(name="tiny", bufs=4))
            ps_pre = ctx.enter_context(tc.tile_pool(name="ps_pre", bufs=3, space="PSUM"))
            ps_st = ctx.enter_context(tc.tile_pool(name="ps_st", bufs=2, space="PSUM"))
            ps_bc = ctx.enter_context(tc.tile_pool(name="ps_bc", bufs=2, space="PSUM"))
            ps_l = ctx.enter_context(tc.tile_pool(name="ps_l", bufs=1, space="PSUM"))
            ps_bp = ps_pre

            # ---- constants / weights into SBUF ----
            wemb_sb = const.tile([P, 3, H], bf16)
            nc.sync.dma_start(out=wemb_sb, in_=wemb_d.ap())
            wx0_sb = const.tile([P, KC, H], bf16)
            nc.sync.dma_start(out=wx0_sb, in_=wx0_d.ap())
            wrec_sb = const.tile([P, 5, KC, H], bf16)
            nc.sync.dma_start(out=wrec_sb, in_=wrec_d.ap().rearrange("n p k m -> p n k m"))
            bias0_sb = const.tile([P, KC], f32)
            nc.sync.dma_start(out=bias0_sb, in_=bias0_d.ap())
            wfc_sb = const.tile([P, KC, NCLS], bf16)
            nc.sync.dma_start(out=wfc_sb, in_=wfc_d.ap())
            fcb_sb = const.tile([NCLS, 1], f32)
            nc.sync.dma_start(out=fcb_sb, in_=fcb_d.ap())
            if apply_gb:
                gb_sb = const.tile([P, 3, 2, KC], f32)
                nc.sync.dma_start(out=gb_sb, in_=gb_d.ap())

            ones16 = const.tile([1, P], f16)
            nc.vector.memset(ones16, 1.0)
            sc_ones = const.tile([P, 1], f16)
            nc.vector.memset(sc_ones, 1.0 / H)
            ones_n = const.tile([1, B], f16)
            nc.vector.memset(ones_n, 1.0)
            eps_row = const.tile([1, B], f16)
            nc.vector.memset(eps_row, LN_EPS * H)
            qk32 = const.tile([B, 1], mybir.dt.int32)
            nc.vector.memset(qk32, 0x5F3759DF)
            eye32_sb = const.tile([B, B], f32)
            nc.sync.dma_start(out=eye32_sb, in_=eye_d.ap())
            brow_sb = const.tile([1, 2, KC, P], f16)
            nc.sync.dma_start(out=brow_sb, in_=brow_d.ap())

            xp_sb = big.tile([P, T, KC, B], bf16)     # xproj0 (+bias0), all steps
            L_sb = big.tile([NCLS, COLS], f32)        # FC accumulator

            # ---- pre-phase: embedding + xproj0, slab by slab ----
            for sl in range(n_slabs):
                c0 = sl * S
                xt_tiles = []
                for k in range(3):
                    xt_t = xtp.tile([P, S], bf16, tag=f"xt{k}")
                    nc.sync.dma_start(out=xt_t, in_=xt_d.ap()[k, :, c0:c0 + S])
                    xt_tiles.append(xt_t)
                xe_tiles = []
                for m in range(KC):
                    xe_t = xep.tile([P, S], bf16, tag=f"xe{m}")
                    xe_tiles.append(xe_t)
                for m in range(KC):
                    for ns in range(S // MMN):
                        pse = ps_bp.tile([P, MMN], f32, tag="pre")
                        for k in range(3):
                            nc.tensor.matmul(pse, wemb_sb[:, k, bass.ts(m, P)],
                                             xt_tiles[k][:, bass.ts(ns, MMN)],
                                             start=(k == 0), stop=(k == 2))
                        nc.scalar.copy(xe_tiles[m][:, bass.ts(ns, MMN)], pse)
                for m in range(KC):
                    for ns in range(S // MMN):
                        psx = ps_bp.tile([P, MMN], f32, tag="pre")
                        for k in range(KC):
                            nc.tensor.matmul(psx, wx0_sb[:, k, bass.ts(m, P)],
                                             xe_tiles[k][:, bass.ts(ns, MMN)],
                                             start=(k == 0), stop=(k == KC - 1))
                        n0 = c0 + ns * MMN
                        t0 = n0 // B
                        nt = MMN // B
                        nc.vector.tensor_scalar(
                            xp_sb[:, t0:t0 + nt, m, :], psx,
                            bias0_sb[:, m:m + 1], None, mybir.AluOpType.add)

            # ---- recurrence (wavefront emission) ----
            h = []
            for l in range(3):
                h0 = hp.tile([P, KC, B], bf16, tag=f"h{l}")
                nc.vector.memset(h0, 0.0)
                h.append(h0)

            wh_idx = [0, 2, 4]   # Wh0, Wh1, Wh2 in wrec
            wx_idx = [None, 1, 3]
            i32 = mybir.dt.int32
            Sq = mybir.ActivationFunctionType.Square

            def emit_unit(t, l):
                ps = ps_pre.tile([P, KC, B], f32, tag="pre")
                for m in range(KC):
                    n_mm = KC * (2 if l > 0 else 1) + (1 if l > 0 else 0)
                    i = 0
                    for k in range(KC):
                        nc.tensor.matmul(ps[:, m, :],
                                         wrec_sb[:, wh_idx[l], k, bass.ts(m, P)],
                                         h[l][:, k, :],
                                         start=(i == 0), stop=(i == n_mm - 1))
                        i += 1
                    if l > 0:
                        for k in range(KC):
                            nc.tensor.matmul(ps[:, m, :],
                                             wrec_sb[:, wx_idx[l], k, bass.ts(m, P)],
                                             h[l - 1][:, k, :],
                                             start=False, stop=(i == n_mm - 1))
                            i += 1
                        nc.tensor.matmul(ps[:, m, :], brow_sb[:, l - 1, m, :],
                                         ones_n, start=False, stop=True)

                st = stp.tile([P, KC, 2 * B], bf16, tag="st")
                if l == 0:
                    nc.vector.tensor_tensor(st[:, :, :B], ps, xp_sb[:, t, :, :],
                                            mybir.AluOpType.add)
                    nc.scalar.activation(st[:, :, B:], st[:, :, :B], Sq)
                else:
                    nc.scalar.copy(st[:, :, :B], ps)
                    nc.scalar.activation(st[:, :, B:], ps, Sq)

                pst = ps_st.tile([B, 2], f32, tag="pst")
                for k in range(KC):
                    nc.tensor.matmul(pst[:, 0:1], st[:, k, :B], sc_ones,
                                     start=(k == 0), stop=(k == KC - 1))
                for k in range(KC):
                    nc.tensor.matmul(pst[:, 1:2], st[:, k, B:], sc_ones,
                                     start=(k == 0), stop=False)
                nc.tensor.matmul(pst[:, 1:2], eps_row, sc_ones[0:1, :],
                                 start=False, stop=True)

                m2 = tiny.tile([B, 1], f32, tag="m2")
                nc.scalar.activation(m2, pst[:, 0:1], Sq)
                ve = tiny.tile([B, 1], f32, tag="ve")
                nc.vector.tensor_tensor(ve, pst[:, 1:2], m2,
                                        mybir.AluOpType.subtract)
                ui = tiny.tile([B, 1], i32, tag="ui")
                nc.vector.tensor_scalar(ui, ve.bitcast(i32), 1, None,
                                        mybir.AluOpType.arith_shift_right)
                y0i = tiny.tile([B, 1], i32, tag="y0i")
                nc.vector.tensor_tensor(y0i, qk32, ui, mybir.AluOpType.subtract)
                cur = y0i.bitcast(f32)
                rc = tiny.tile([B, 2], f32, tag="rc")
                for it in range(2):
                    y2 = tiny.tile([B, 1], f32, tag=f"nw_y2_{it}")
                    nc.vector.tensor_tensor(y2, cur, cur, mybir.AluOpType.mult)
                    xy2 = tiny.tile([B, 1], f32, tag=f"nw_xy2_{it}")
                    nc.vector.tensor_tensor(xy2, ve, y2, mybir.AluOpType.mult)
                    e = tiny.tile([B, 1], f32, tag=f"nw_e_{it}")
                    nc.vector.tensor_scalar(e, xy2, -0.5, 1.5,
                                            mybir.AluOpType.mult,
                                            mybir.AluOpType.add)
                    if it == 0:
                        yn = tiny.tile([B, 1], f32, tag="nw_yn")
                        nc.vector.tensor_tensor(yn, cur, e, mybir.AluOpType.mult)
                        cur = yn
                    else:
                        nc.vector.tensor_tensor(rc[:, 0:1], cur, e,
                                                mybir.AluOpType.mult)
                nc.vector.tensor_tensor(rc[:, 1:2], pst[:, 0:1], rc[:, 0:1],
                                        mybir.AluOpType.mult)

                bctr = ps_bc.tile([P, 4 * B], f32, tag="bc")
                nc.tensor.transpose(bctr[0:1, 2 * B:3 * B], rc[:, 0:1], eye32_sb)
                nc.tensor.transpose(bctr[0:1, 3 * B:4 * B], rc[:, 1:2], eye32_sb)
                rcrow = tiny.tile([1, 2 * B], f16, tag="rcrow")
                nc.scalar.copy(rcrow, bctr[0:1, 2 * B:4 * B])
                nc.tensor.matmul(bctr[:, 0:2 * B], ones16, rcrow,
                                 start=True, stop=True)

                y = yp.tile([P, KC, B], f32, tag="y")
                nc.vector.tensor_tensor(y, st[:, :, :B],
                                        _stride0_view(bctr[:, 0:B], KC, B),
                                        mybir.AluOpType.mult)
                nc.vector.tensor_tensor(y, y,
                                        _stride0_view(bctr[:, B:2 * B], KC, B),
                                        mybir.AluOpType.subtract)
                if apply_gb:
                    nc.vector.tensor_tensor(
                        y, y, _stride0_view_mid(gb_sb[:, l, 0, :], B),
                        mybir.AluOpType.mult)
                    nc.vector.tensor_tensor(
                        y, y, _stride0_view_mid(gb_sb[:, l, 1, :], B),
                        mybir.AluOpType.add)
                hn = hp.tile([P, KC, B], bf16, tag=f"h{l}")
                nc.scalar.activation(hn, y, mybir.ActivationFunctionType.Tanh)
                h[l] = hn
                if l == 2:
                    psl = ps_l.tile([NCLS, B], f32, tag="L")
                    for k in range(KC):
                        nc.tensor.matmul(psl, wfc_sb[:, k, :], h[2][:, k, :],
                                         start=(k == 0), stop=(k == KC - 1))
                    nc.vector.tensor_scalar(L_sb[:, t * B:(t + 1) * B], psl,
                                            fcb_sb, None, mybir.AluOpType.add)

            for tau in range(T + 2):
                for l in (2, 1, 0):
                    t = tau - l
                    if 0 <= t < T:
                        emit_unit(t, l)

            nc.sync.dma_start(out=out_d.ap(), in_=L_sb)

    nc.compile()
    return nc


def _stride0_view_mid(ap, width):
    """[P, KC] AP -> [P, KC, width] AP, broadcasting each col along width."""
    return bass.AP(tensor=ap.tensor, offset=ap.offset,
                   ap=[ap.ap[0], ap.ap[1], [0, width]])


# ---------------- host-side prep ----------------

def _lay_w(w):
    """[H, M] fp32 -> [P, KC, M] bf16 chunk layout."""
    Hh, M = w.shape
    kc = Hh // P
    return np.ascontiguousarray(
        w.reshape(kc, P, M).transpose(1, 0, 2)).astype(NP16)


def make_in_maps(inputs, T=256, B=32):
    """Build the 8 per-core input dicts from the full problem inputs."""
    x = np.asarray(inputs["x"], np.float32)[:, :T]
    rx = np.asarray(inputs["reverse_x"], np.float32)[:, :T]
    W_emb = np.asarray(inputs["W_emb"], np.float32)
    b_emb = np.asarray(inputs["b_emb"], np.float32)
    W_fc = np.asarray(inputs["W_fc"], np.float32)
    b_fc = np.asarray(inputs["b_fc"], np.float32)

    wemb_aug = np.zeros((3 * P, H), np.float32)
    wemb_aug[:IN_DIM] = W_emb
    wemb_aug[IN_DIM] = b_emb
    wemb_lay = _lay_w(wemb_aug)  # [P, 3, H]

    dirs = {}
    for d, (xx, sfx, wfc_half, fcb) in enumerate([
            (x, "l2r", W_fc[:H], b_fc),
            (rx, "r2l", W_fc[H:], np.zeros_like(b_fc))]):
        Wx = np.asarray(inputs[f"Wx_{sfx}"], np.float32)
        bx = np.asarray(inputs[f"bx_{sfx}"], np.float32)
        Wh = np.asarray(inputs[f"Wh_{sfx}"], np.float32)
        bh = np.asarray(inputs[f"bh_{sfx}"], np.float32)
        wrec = np.stack([_lay_w(Wh[0]), _lay_w(Wx[1]), _lay_w(Wh[1]),
                         _lay_w(Wx[2]), _lay_w(Wh[2])])  # [5, P, KC, H]
        bias0 = (bx[0] + bh[0]).reshape(KC, P).T.astype(np.float32)  # [P, KC]
        brow = np.stack([(bx[1] + bh[1]).reshape(KC, P),
                         (bx[2] + bh[2]).reshape(KC, P)])[None].astype(np.float16)
        dirs[d] = dict(
            x=xx,
            wx0=_lay_w(Wx[0]),
            wrec=np.ascontiguousarray(wrec),
            bias0=np.ascontiguousarray(bias0),
            brow=np.ascontiguousarray(brow),
            wfc=_lay_w(wfc_half),
            fcb=fcb.reshape(NCLS, 1).astype(np.float32),
        )

    n_shard = N_CORES // 2
    in_maps = []
    for core in range(N_CORES):
        d = 0 if core < n_shard else 1
        s = core % n_shard
        dd = dirs[d]
        xc = dd["x"][s * B:(s + 1) * B]  # [B, T, IN]
        xa = np.zeros((3 * P, T * B), np.float32)
        xa[:IN_DIM] = xc.transpose(2, 1, 0).reshape(IN_DIM, T * B)
        xa[IN_DIM] = 1.0
        in_maps.append({
            "xt": np.ascontiguousarray(xa.reshape(3, P, T * B)).astype(NP16),
            "wemb": wemb_lay,
            "wx0": dd["wx0"],
            "wrec": dd["wrec"],
            "bias0": dd["bias0"],
            "brow": dd["brow"],
            "eye32": np.eye(B, dtype=np.float32),
            "wfc": dd["wfc"],
            "fcb": dd["fcb"],
        })
    return in_maps


def combine_outputs(results, pad_start_index, T=256, B=32):
    """results: list of 8 dicts with 'out' [NCLS, T*B]. Returns [128*T, NCLS]."""
    n_shard = N_CORES // 2
    Bfull = n_shard * B
    L = np.zeros((2, Bfull, T, NCLS), np.float32)
    for core in range(N_CORES):
        d = 0 if core < n_shard else 1
        s = core % n_shard
        o = results[core]["out"].reshape(NCLS, T, B)  # col = t*B + b
        L[d, s * B:(s + 1) * B] = o.transpose(2, 1, 0)
    p = np.asarray(pad_start_index).astype(np.int64)[:, None]
    j = np.arange(T)[None, :]
    idx = np.where(j < p, p - j - 1, j)  # [Bfull, T]
    L2g = np.take_along_axis(L[1], idx[:, :, None], axis=1)
    logits = L[0] + L2g
    return logits.reshape(Bfull * T, NCLS)


_NC_CACHE = {}


def kernel(**inputs) -> np.ndarray:
    T = int(inputs["max_length"])
    assert T == 256, f"kernel compiled for T=256, got {T}"
    B = 32
    ln_g = np.asarray(inputs["ln_g"], np.float32)
    ln_b = np.asarray(inputs["ln_b"], np.float32)
    apply_gb = not (np.all(ln_g == 1.0) and np.all(ln_b == 0.0))
    assert not apply_gb, "general ln_g/ln_b path not wired yet"

    key = (T, B, apply_gb)
    if key not in _NC_CACHE:
        _NC_CACHE[key] = build_nc(T=T, B=B, apply_gb=apply_gb)
    nc = _NC_CACHE[key]

    in_maps = make_in_maps(inputs, T=T, B=B)
    res = run_bass_kernel_spmd(nc, in_maps, list(range(N_CORES)))
    return combine_outputs(res.results, inputs["pad_start_index"], T=T, B=B)


if __name__ == "__main__":
    import reference
    inp = reference.setup_inputs()
    out = kernel(**{k: np.asarray(v) for k, v in inp.items()})
    ref = np.asarray(reference.reference(**inp))
    err = np.abs(out - ref).max() / np.abs(ref).max()
    print(f"Relative error: {err:.3e}")


# revision 7
# speedup vs baseline: 1.5284x; 1.2721x over previous
"""Trainium2 Bass kernel for nn_BidirectionalRNN (3-layer LN-tanh RNN, bidir).

Sharding: 8 cores = 4 batch-shards x 2 directions (l2r on cores 0-3, r2l on
cores 4-7), B_loc=32 batches per core. All cores run the same SPMD program;
per-core inputs differ (direction weights + batch slice).

On-core layout: everything "transposed" — H on partitions as 4 chunks of 128,
batch along free dim. State h_l is one SBUF tile [128, 4, 32] (chunk-major).
Per step/layer:
  psum_pre[128,4,32] = Wh_l.T @ h_l(t-1) + Wx_l.T @ h_{l-1}(t)   (bf16 matmuls)
  s  = psum_pre + bias  (layer0: + xproj[t], bias prefolded)  -> bf16 st tile
  s2 = s*s                                                     -> st cols B:2B
  stats[1,2B] = (ones/512).T @ st  (PE, per k-chunk accumulate)  = [mean|meansq]
  m2 = Square(mean) (ACT); var = meansq - m2 (DVE)
  rstd = pow(var + eps, -0.5) (DVE tensor_scalar dual-op); c = mean*rstd
  [A|C][128,2B] = ones16.T @ [rstd|c]  (PE broadcast, fp16)
  y = s*A - C ; h_l = Tanh(y) (ACT, out bf16)
Embedding + xproj0 for layer0 are precomputed into SBUF (xp tile, bf16) by a
slab-wise pre-phase: xemb.T = Wemb_aug.T @ xT_aug (bias via appended ones row),
xproj = Wx0.T @ xemb.T + (bx0+bh0).
Final FC is accumulated per step on the PE: L[45, t*B:(t+1)*B] = Wfc_half.T @ h2
(+ b_fc on the l2r cores only, via per-core input).
Host combines: logits[b,t] = L_l2r[b,t] + L_r2l[b, idx[b,t]] (time gather
commutes with the channel-wise FC).
"""

import numpy as np
import ml_dtypes

import concourse.bass as bass
import concourse.bacc as bacc
import concourse.tile as tile
from concourse import mybir
from concourse.bass_utils import run_bass_kernel_spmd

BF16 = ml_dtypes.bfloat16
import os
USE_FP16 = os.environ.get("KERNEL_DT16", "f16") == "f16"
N_NEWTON = int(os.environ.get("KERNEL_NEWTON", "1"))
NP16 = np.float16 if USE_FP16 else BF16

H = 512
IN_DIM = 300
NCLS = 45
LN_EPS = 1e-5
P = 128
KC = H // P  # 4 chunks
N_CORES = 8

f32 = mybir.dt.float32
f16 = mybir.dt.float16
bf16 = mybir.dt.float16 if USE_FP16 else mybir.dt.bfloat16


def _stride0_view(ap, reps, width):
    """[P, width] AP -> [P, reps, width] AP re-reading the same cols."""
    return bass.AP(tensor=ap.tensor, offset=ap.offset,
                   ap=[ap.ap[0], [0, reps], [1, width]])


def build_nc(T=256, B=32, apply_gb=False):
    """Build the SPMD program. B = batches per core."""
    COLS = T * B
    S = min(1024, COLS)          # pre-phase slab width (cols)
    MMN = min(512, S)            # matmul moving width
    n_slabs = COLS // S

    nc = bacc.Bacc(None, target_bir_lowering=False)

    # ---- DRAM parameters (per-core values supplied via in_maps) ----
    xt_d = nc.dram_tensor("xt", [3, P, COLS], bf16, kind="ExternalInput")
    wemb_d = nc.dram_tensor("wemb", [P, 3, H], bf16, kind="ExternalInput")
    wx0_d = nc.dram_tensor("wx0", [P, KC, H], bf16, kind="ExternalInput")
    # recurrence weights: Wh0, Wx1, Wh1, Wx2, Wh2
    wrec_d = nc.dram_tensor("wrec", [5, P, KC, H], bf16, kind="ExternalInput")
    bias0_d = nc.dram_tensor("bias0", [P, KC], f32, kind="ExternalInput")
    bias12_d = nc.dram_tensor("bias12", [P, 2, KC], f32, kind="ExternalInput")
    eye_d = nc.dram_tensor("eye32", [B, B], f32, kind="ExternalInput")
    wfc_d = nc.dram_tensor("wfc", [P, KC, NCLS], bf16, kind="ExternalInput")
    fcb_d = nc.dram_tensor("fcb", [NCLS, 1], f32, kind="ExternalInput")
    if apply_gb:
        gb_d = nc.dram_tensor("gb", [P, 3, 2, KC], f32, kind="ExternalInput")
    out_d = nc.dram_tensor("out", [NCLS, COLS], f32, kind="ExternalOutput")

    with tile.TileContext(nc) as tc:
        import contextlib
        with contextlib.ExitStack() as ctx:
            const = ctx.enter_context(tc.tile_pool(name="const", bufs=1))
            big = ctx.enter_context(tc.tile_pool(name="big", bufs=1))
            xtp = ctx.enter_context(tc.tile_pool(name="xtp", bufs=2))
            xep = ctx.enter_context(tc.tile_pool(name="xep", bufs=2))
            stp = ctx.enter_context(tc.tile_pool(name="stp", bufs=3))
            hp = ctx.enter_context(tc.tile_pool(name="hp", bufs=3))
            yp = ctx.enter_context(tc.tile_pool(name="yp", bufs=3))
            tiny = ctx.enter_context(tc.tile_pool(name="tiny", bufs=4))
            ps_pre = ctx.enter_context(tc.tile_pool(name="ps_pre", bufs=3, space="PSUM"))
            ps_st = ctx.enter_context(tc.tile_pool(name="ps_st", bufs=2, space="PSUM"))
            ps_bc = ctx.enter_context(tc.tile_pool(name="ps_bc", bufs=2, space="PSUM"))
            ps_l = ctx.enter_context(tc.tile_pool(name="ps_l", bufs=1, space="PSUM"))
            ps_bp = ps_pre

            # ---- constants / weights into SBUF ----
            wemb_sb = const.tile([P, 3, H], bf16)
            nc.sync.dma_start(out=wemb_sb, in_=wemb_d.ap())
            wx0_sb = const.tile([P, KC, H], bf16)
            nc.sync.dma_start(out=wx0_sb, in_=wx0_d.ap())
            wrec_sb = const.tile([P, 5, KC, H], bf16)
            nc.sync.dma_start(out=wrec_sb, in_=wrec_d.ap().rearrange("n p k m -> p n k m"))
            bias0_sb = const.tile([P, KC], f32)
            nc.sync.dma_start(out=bias0_sb, in_=bias0_d.ap())
            wfc_sb = const.tile([P, KC, NCLS], bf16)
            nc.sync.dma_start(out=wfc_sb, in_=wfc_d.ap())
            fcb_sb = const.tile([NCLS, 1], f32)
            nc.sync.dma_start(out=fcb_sb, in_=fcb_d.ap())
            if apply_gb:
                gb_sb = const.tile([P, 3, 2, KC], f32)
                nc.sync.dma_start(out=gb_sb, in_=gb_d.ap())

            ones16 = const.tile([1, P], f16)
            nc.vector.memset(ones16, 1.0)
            sc_ones = const.tile([P, 1], f16)
            nc.vector.memset(sc_ones, 1.0 / H)
            ones_n = const.tile([1, B], f16)
            nc.vector.memset(ones_n, 1.0)
            eps_row = const.tile([1, B], f16)
            nc.vector.memset(eps_row, LN_EPS * H)
            qk32 = const.tile([B, 1], mybir.dt.int32)
            nc.vector.memset(qk32, 0x5F3759DF)
            eye32_sb = const.tile([B, B], f32)
            nc.sync.dma_start(out=eye32_sb, in_=eye_d.ap())
            bias12_sb = const.tile([P, 2, KC], f32)
            nc.sync.dma_start(out=bias12_sb, in_=bias12_d.ap())

            xp_sb = big.tile([P, T, KC, B], bf16)     # xproj0 (+bias0), all steps
            L_sb = big.tile([NCLS, COLS], f32)        # FC accumulator

            # ---- pre-phase: embedding + xproj0, slab by slab ----
            for sl in range(n_slabs):
                c0 = sl * S
                xt_tiles = []
                for k in range(3):
                    xt_t = xtp.tile([P, S], bf16, tag=f"xt{k}")
                    nc.sync.dma_start(out=xt_t, in_=xt_d.ap()[k, :, c0:c0 + S])
                    xt_tiles.append(xt_t)
                xe_tiles = []
                for m in range(KC):
                    xe_t = xep.tile([P, S], bf16, tag=f"xe{m}")
                    xe_tiles.append(xe_t)
                for m in range(KC):
                    for ns in range(S // MMN):
                        pse = ps_bp.tile([P, MMN], f32, tag="pre")
                        for k in range(3):
                            nc.tensor.matmul(pse, wemb_sb[:, k, bass.ts(m, P)],
                                             xt_tiles[k][:, bass.ts(ns, MMN)],
                                             start=(k == 0), stop=(k == 2))
                        nc.scalar.copy(xe_tiles[m][:, bass.ts(ns, MMN)], pse)
                for m in range(KC):
                    for ns in range(S // MMN):
                        psx = ps_bp.tile([P, MMN], f32, tag="pre")
                        for k in range(KC):
                            nc.tensor.matmul(psx, wx0_sb[:, k, bass.ts(m, P)],
                                             xe_tiles[k][:, bass.ts(ns, MMN)],
                                             start=(k == 0), stop=(k == KC - 1))
                        n0 = c0 + ns * MMN
                        t0 = n0 // B
                        nt = MMN // B
                        nc.vector.tensor_scalar(
                            xp_sb[:, t0:t0 + nt, m, :], psx,
                            bias0_sb[:, m:m + 1], None, mybir.AluOpType.add)

            # ---- recurrence (wavefront emission) ----
            h = []
            for l in range(3):
                h0 = hp.tile([P, KC, B], bf16, tag=f"h{l}")
                nc.vector.memset(h0, 0.0)
                h.append(h0)

            wh_idx = [0, 2, 4]   # Wh0, Wh1, Wh2 in wrec
            wx_idx = [None, 1, 3]
            i32 = mybir.dt.int32
            Sq = mybir.ActivationFunctionType.Square

            def emit_unit(t, l):
                ps = ps_pre.tile([P, KC, B], f32, tag="pre")
                for m in range(KC):
                    n_mm = KC * (2 if l > 0 else 1)
                    i = 0
                    for k in range(KC):
                        nc.tensor.matmul(ps[:, m, :],
                                         wrec_sb[:, wh_idx[l], k, bass.ts(m, P)],
                                         h[l][:, k, :],
                                         start=(i == 0), stop=(i == n_mm - 1))
                        i += 1
                    if l > 0:
                        for k in range(KC):
                            nc.tensor.matmul(ps[:, m, :],
                                             wrec_sb[:, wx_idx[l], k, bass.ts(m, P)],
                                             h[l - 1][:, k, :],
                                             start=False, stop=(i == n_mm - 1))
                            i += 1

                st = stp.tile([P, KC, 2 * B], bf16, tag="st")
                if l == 0:
                    nc.vector.tensor_tensor(st[:, :, :B], ps, xp_sb[:, t, :, :],
                                            mybir.AluOpType.add)
                else:
                    nc.vector.tensor_tensor(
                        st[:, :, :B], ps,
                        _stride0_view_mid(bias12_sb[:, l - 1, :], B),
                        mybir.AluOpType.add)
                nc.scalar.activation(st[:, :, B:], st[:, :, :B], Sq)

                pst = ps_st.tile([B, 2], f32, tag="pst")
                for k in range(KC):
                    nc.tensor.matmul(pst[:, 0:1], st[:, k, :B], sc_ones,
                                     start=(k == 0), stop=(k == KC - 1))
                for k in range(KC):
                    nc.tensor.matmul(pst[:, 1:2], st[:, k, B:], sc_ones,
                                     start=(k == 0), stop=(k == KC - 1))

                m2 = tiny.tile([B, 1], f32, tag="m2")
                nc.scalar.activation(m2, pst[:, 0:1], Sq)
                ve = tiny.tile([B, 1], f32, tag="ve")
                nc.vector.tensor_tensor(ve, pst[:, 1:2], m2,
                                        mybir.AluOpType.subtract)
                nc.vector.tensor_scalar(ve, ve, LN_EPS, None,
                                        mybir.AluOpType.add)
                ui = tiny.tile([B, 1], i32, tag="ui")
                nc.vector.tensor_scalar(ui, ve.bitcast(i32), 1, None,
                                        mybir.AluOpType.arith_shift_right)
                y0i = tiny.tile([B, 1], i32, tag="y0i")
                nc.vector.tensor_tensor(y0i, qk32, ui, mybir.AluOpType.subtract)
                cur = y0i.bitcast(f32)
                rc = tiny.tile([B, 2], f32, tag="rc")
                for it in range(N_NEWTON):
                    y2 = tiny.tile([B, 1], f32, tag=f"nw_y2_{it}")
                    nc.vector.tensor_tensor(y2, cur, cur, mybir.AluOpType.mult)
                    xy2 = tiny.tile([B, 1], f32, tag=f"nw_xy2_{it}")
                    nc.vector.tensor_tensor(xy2, ve, y2, mybir.AluOpType.mult)
                    e = tiny.tile([B, 1], f32, tag=f"nw_e_{it}")
                    nc.vector.tensor_scalar(e, xy2, -0.5, 1.5,
                                            mybir.AluOpType.mult,
                                            mybir.AluOpType.add)
                    if it < N_NEWTON - 1:
                        yn = tiny.tile([B, 1], f32, tag=f"nw_yn_{it}")
                        nc.vector.tensor_tensor(yn, cur, e, mybir.AluOpType.mult)
                        cur = yn
                    else:
                        nc.vector.tensor_tensor(rc[:, 0:1], cur, e,
                                                mybir.AluOpType.mult)
                nc.vector.tensor_tensor(rc[:, 1:2], pst[:, 0:1], rc[:, 0:1],
                                        mybir.AluOpType.mult)

                bctr = ps_bc.tile([P, 4 * B], f32, tag="bc")
                nc.tensor.transpose(bctr[0:1, 2 * B:3 * B], rc[:, 0:1], eye32_sb)
                nc.tensor.transpose(bctr[0:1, 3 * B:4 * B], rc[:, 1:2], eye32_sb)
                rcrow = tiny.tile([1, 2 * B], f16, tag="rcrow")
                nc.scalar.copy(rcrow, bctr[0:1, 2 * B:4 * B])
                nc.tensor.matmul(bctr[:, 0:2 * B], ones16, rcrow,
                                 start=True, stop=True)

                ac_sb = tiny.tile([P, 2 * B], f16, tag="ac_sb")
                nc.scalar.copy(ac_sb, bctr[:, 0:2 * B])
                y = yp.tile([P, KC, B], f16, tag="y")
                nc.vector.tensor_tensor(y, st[:, :, :B],
                                        _stride0_view(ac_sb[:, 0:B], KC, B),
                                        mybir.AluOpType.mult)
                nc.vector.tensor_tensor(y, y,
                                        _stride0_view(ac_sb[:, B:2 * B], KC, B),
                                        mybir.AluOpType.subtract)
                if apply_gb:
                    nc.vector.tensor_tensor(
                        y, y, _stride0_view_mid(gb_sb[:, l, 0, :], B),
                        mybir.AluOpType.mult)
                    nc.vector.tensor_tensor(
                        y, y, _stride0_view_mid(gb_sb[:, l, 1, :], B),
                        mybir.AluOpType.add)
                hn = hp.tile([P, KC, B], bf16, tag=f"h{l}")
                nc.scalar.activation(hn, y, mybir.ActivationFunctionType.Tanh)
                h[l] = hn
                if l == 2:
                    psl = ps_l.tile([NCLS, B], f32, tag="L")
                    for k in range(KC):
                        nc.tensor.matmul(psl, wfc_sb[:, k, :], h[2][:, k, :],
                                         start=(k == 0), stop=(k == KC - 1))
                    nc.vector.tensor_scalar(L_sb[:, t * B:(t + 1) * B], psl,
                                            fcb_sb, None, mybir.AluOpType.add)

            for tau in range(T + 2):
                for l in (2, 1, 0):
                    t = tau - l
                    if 0 <= t < T:
                        emit_unit(t, l)

            nc.sync.dma_start(out=out_d.ap(), in_=L_sb)

    nc.compile()
    return nc


def _stride0_view_mid(ap, width):
    """[P, KC] AP -> [P, KC, width] AP, broadcasting each col along width."""
    return bass.AP(tensor=ap.tensor, offset=ap.offset,
                   ap=[ap.ap[0], ap.ap[1], [0, width]])


# ---------------- host-side prep ----------------

def _lay_w(w):
    """[H, M] fp32 -> [P, KC, M] bf16 chunk layout."""
    Hh, M = w.shape
    kc = Hh // P
    return np.ascontiguousarray(
        w.reshape(kc, P, M).transpose(1, 0, 2)).astype(NP16)


def make_in_maps(inputs, T=256, B=32):
    """Build the 8 per-core input dicts from the full problem inputs."""
    x = np.asarray(inputs["x"], np.float32)[:, :T]
    rx = np.asarray(inputs["reverse_x"], np.float32)[:, :T]
    W_emb = np.asarray(inputs["W_emb"], np.float32)
    b_emb = np.asarray(inputs["b_emb"], np.float32)
    W_fc = np.asarray(inputs["W_fc"], np.float32)
    b_fc = np.asarray(inputs["b_fc"], np.float32)

    wemb_aug = np.zeros((3 * P, H), np.float32)
    wemb_aug[:IN_DIM] = W_emb
    wemb_aug[IN_DIM] = b_emb
    wemb_lay = _lay_w(wemb_aug)  # [P, 3, H]

    dirs = {}
    for d, (xx, sfx, wfc_half, fcb) in enumerate([
            (x, "l2r", W_fc[:H], b_fc),
            (rx, "r2l", W_fc[H:], np.zeros_like(b_fc))]):
        Wx = np.asarray(inputs[f"Wx_{sfx}"], np.float32)
        bx = np.asarray(inputs[f"bx_{sfx}"], np.float32)
        Wh = np.asarray(inputs[f"Wh_{sfx}"], np.float32)
        bh = np.asarray(inputs[f"bh_{sfx}"], np.float32)
        wrec = np.stack([_lay_w(Wh[0]), _lay_w(Wx[1]), _lay_w(Wh[1]),
                         _lay_w(Wx[2]), _lay_w(Wh[2])])  # [5, P, KC, H]
        bias0 = (bx[0] + bh[0]).reshape(KC, P).T.astype(np.float32)  # [P, KC]
        bias12 = np.stack([(bx[1] + bh[1]).reshape(KC, P).T,
                           (bx[2] + bh[2]).reshape(KC, P).T], 1).astype(np.float32)
        dirs[d] = dict(
            x=xx,
            wx0=_lay_w(Wx[0]),
            wrec=np.ascontiguousarray(wrec),
            bias0=np.ascontiguousarray(bias0),
            bias12=np.ascontiguousarray(bias12),
            wfc=_lay_w(wfc_half),
            fcb=fcb.reshape(NCLS, 1).astype(np.float32),
        )

    n_shard = N_CORES // 2
    in_maps = []
    for core in range(N_CORES):
        d = 0 if core < n_shard else 1
        s = core % n_shard
        dd = dirs[d]
        xc = dd["x"][s * B:(s + 1) * B]  # [B, T, IN]
        xa = np.zeros((3 * P, T * B), np.float32)
        xa[:IN_DIM] = xc.transpose(2, 1, 0).reshape(IN_DIM, T * B)
        xa[IN_DIM] = 1.0
        in_maps.append({
            "xt": np.ascontiguousarray(xa.reshape(3, P, T * B)).astype(NP16),
            "wemb": wemb_lay,
            "wx0": dd["wx0"],
            "wrec": dd["wrec"],
            "bias0": dd["bias0"],
            "bias12": dd["bias12"],
            "eye32": np.eye(B, dtype=np.float32),
            "wfc": dd["wfc"],
            "fcb": dd["fcb"],
        })
    return in_maps


def combine_outputs(results, pad_start_index, T=256, B=32):
    """results: list of 8 dicts with 'out' [NCLS, T*B]. Returns [128*T, NCLS]."""
    n_shard = N_CORES // 2
    Bfull = n_shard * B
    L = np.zeros((2, Bfull, T, NCLS), np.float32)
    for core in range(N_CORES):
        d = 0 if core < n_shard else 1
        s = core % n_shard
        o = results[core]["out"].reshape(NCLS, T, B)  # col = t*B + b
        L[d, s * B:(s + 1) * B] = o.transpose(2, 1, 0)
    p = np.asarray(pad_start_index).astype(np.int64)[:, None]
    j = np.arange(T)[None, :]
    idx = np.where(j < p, p - j - 1, j)  # [Bfull, T]
    L2g = np.take_along_axis(L[1], idx[:, :, None], axis=1)
    logits = L[0] + L2g
    return logits.reshape(Bfull * T, NCLS)


_NC_CACHE = {}


def kernel(**inputs) -> np.ndarray:
    T = int(inputs["max_length"])
    assert T == 256, f"kernel compiled for T=256, got {T}"
    B = 32
    ln_g = np.asarray(inputs["ln_g"], np.float32)
    ln_b = np.asarray(inputs["ln_b"], np.float32)
    apply_gb = not (np.all(ln_g == 1.0) and np.all(ln_b == 0.0))
    assert not apply_gb, "general ln_g/ln_b path not wired yet"

    key = (T, B, apply_gb)
    if key not in _NC_CACHE:
        _NC_CACHE[key] = build_nc(T=T, B=B, apply_gb=apply_gb)
    nc = _NC_CACHE[key]

    in_maps = make_in_maps(inputs, T=T, B=B)
    res = run_bass_kernel_spmd(nc, in_maps, list(range(N_CORES)))
    return combine_outputs(res.results, inputs["pad_start_index"], T=T, B=B)


if __name__ == "__main__":
    import reference
    inp = reference.setup_inputs()
    out = kernel(**{k: np.asarray(v) for k, v in inp.items()})
    ref = np.asarray(reference.reference(**inp))
    err = np.abs(out - ref).max() / np.abs(ref).max()
    print(f"Relative error: {err:.3e}")
